# revision 11
# baseline (speedup 1.0000x reference)
"""Causal depthwise Conv1d (B=8, T=4096, C=2048, K=4), fp32, on 8 NeuronCores.

Mode "t3" (default): batch-parallel across 8 cores, fp16 device I/O
(host casts + transposes to [B, C, T]).  Per 128-channel block:

  - PE: taps 0..2 as 32x32 *tiled* diagonal matmuls.  Each 128-wide diag
    matmul is split into its 4 nonzero 32x32 diagonal tiles via
    tile_position=(32g, 32g).  The 4 sub-arrays stream concurrently and
    each LDWEIGHTS is only 32 columns (~27 ns) and can be pulled ahead
    across row groups, removing the ~100 ns/MM serialized weight-reload
    tax of the full-array version (v7: 325 ns/MM -> target ~220 ns/MM
    equivalent).
  - The diagonal lhsT tiles are precomputed on HOST into a packed
    [128, 16*3*32] fp16 tensor (one 32-col strip per (block, tap) with
    the 4 diag tiles stacked per partition group) and DMA'd once --
    this removes the 48 ACT builds of v7.
  - ACT: tap 3 + bias via the activation affine in fp16-out mode
    (2x the fp32 rate), per 2048-col half.
  - DVE: out = psum + y3 (tensor_tensor, fp16 out), per half.
  - tap-3 scale/bias come from a single upfront packed [128, 32] fp32
    DMA (wpack) instead of 32 tiny per-block DMAs.
  - input x DMA per block (1 MB) on the sync HWDGE queue; output store
    per block (1 MB) on the scalar HWDGE queue.

Numerics identical to v7 (fp16 taps, exact PE products, fp32 PSUM)
except y3 is rounded to fp16 before the final add: absmax/scale
~1e-3 vs the 2e-2 budget.

Mode "v7" (previous baseline, kept for A/B): full-array diag matmuls,
ACT-built lhsT, fp32 y3.  ~125.5 us HW; PE-bound on serialized
LDWEIGHTS; also shows an intermittent sparse-error race (~1 in 2 runs
observed absmax/scale 1.6e-1 from a handful of elements).
"""

import os
from contextlib import ExitStack

import numpy as np

import concourse.bacc as bacc
import concourse.bass as bass
import concourse.mybir as mybir
import concourse.tile as tile
from concourse.bass_utils import run_bass_kernel_spmd

B, T, C, K = 8, 4096, 2048, 4
P = 128                 # partitions per channel block
CB = C // P             # 16 channel blocks
TT = 512                # free-dim cols per matmul (one PSUM bank)
HALF = 2048             # free elements per PSUM tile (4 banks)
N_CORES = 8

MODE = os.environ.get("KERNEL_MODE", "t3")

LAST_EXEC_NS = None
LAST_RESULTS = None

_PROGRAM_CACHE = {}
_PROFILING_READY = False


def _setup_profiling():
    """Register the axon NTFF profile hook (the image lacks
    antenv.axon_hooks, so shim it into sys.modules) and neuter the S3
    artifact upload."""
    global _PROFILING_READY
    if _PROFILING_READY:
        return
    import sys
    import types

    if "antenv.axon_hooks" not in sys.modules:
        mod = types.ModuleType("antenv.axon_hooks")
        mod._hook = None

        def set_axon_ntff_profile_hook(h):
            mod._hook = h

        def get_axon_ntff_profile_hook():
            return mod._hook

        mod.set_axon_ntff_profile_hook = set_axon_ntff_profile_hook
        mod.get_axon_ntff_profile_hook = get_axon_ntff_profile_hook
        sys.modules["antenv.axon_hooks"] = mod
        import antenv

        antenv.axon_hooks = mod

    from antenv.axon_hooks import (
        get_axon_ntff_profile_hook,
        set_axon_ntff_profile_hook,
    )

    if get_axon_ntff_profile_hook() is None:
        from trn_agent_boot.trn_boot import _ntff_profile_via_ctypes

        set_axon_ntff_profile_hook(
            _ntff_profile_via_ctypes("/opt/axon/libaxon_pjrt.so")
        )

    import concourse.bass_utils as bu

    bu.upload_artifacts = lambda tmpdir: str(tmpdir)
    _PROFILING_READY = True


def _build_t3() -> bass.Bass:
    f16 = mybir.dt.float16
    nc = bacc.Bacc("TRN2", target_bir_lowering=False, debug=False)

    x_d = nc.dram_tensor("x", [C, T], f16, kind="ExternalInput")
    dpack_d = nc.dram_tensor(
        "dpack", [P, CB * 3 * 32], f16, kind="ExternalInput"
    )
    wpack_d = nc.dram_tensor(
        "wpack", [P, CB * 2], mybir.dt.float32, kind="ExternalInput"
    )
    o_d = nc.dram_tensor("out", [C, T], f16, kind="ExternalOutput")

    with tile.TileContext(nc) as tc, ExitStack() as ctx:
        const_pool = ctx.enter_context(tc.tile_pool(name="const", bufs=1))
        x_pool = ctx.enter_context(tc.tile_pool(name="x", bufs=6))
        out_pool = ctx.enter_context(tc.tile_pool(name="o", bufs=4))
        y_pool = ctx.enter_context(tc.tile_pool(name="y", bufs=3))
        psum_pool = ctx.enter_context(
            tc.tile_pool(name="ps", bufs=2, space="PSUM")
        )

        # dpack/wpack ride the sync HWDGE queue: the SP engine reaches its
        # first DMA right after the preamble barrier, while ACT is blocked
        # ~8 us by LoadActFuncSet table loads.
        dpack_sb = const_pool.tile([P, CB * 3 * 32], f16, tag="dpack")
        nc.sync.dma_start(dpack_sb[:], dpack_d[:])
        wpack_sb = const_pool.tile([P, CB * 2], mybir.dt.float32, tag="wpack")
        nc.sync.dma_start(wpack_sb[:], wpack_d[:])

        for cb in range(CB):
            c0 = cb * P

            # Input x rides SWDGE (gpsimd): its completion increment is
            # chained behind the data writes (write-after-write descriptor),
            # so consumers can't wake before the bytes land.  The HWDGE
            # completion inc was observed to overtake the last in-flight
            # SBUF writes by ~100ns under port contention, letting PE/ACT
            # read the first columns of one partition stale (~1 run in 2,
            # always output t=0..3 of one channel).
            xt = x_pool.tile([P, T + K - 1], f16, tag="x")
            if cb == 0:
                # Split the first block's load so half-0 consumers start
                # ~2.5 us earlier (pipeline fill).
                nc.gpsimd.dma_start(
                    xt[:, K - 1 : K - 1 + HALF], x_d[c0 : c0 + P, 0:HALF]
                )
                nc.gpsimd.dma_start(
                    xt[:, K - 1 + HALF : T + K - 1], x_d[c0 : c0 + P, HALF:T]
                )
            else:
                nc.gpsimd.dma_start(
                    xt[:, K - 1 : T + K - 1], x_d[c0 : c0 + P, :]
                )
            nc.vector.memset(xt[:, 0 : K - 1], 0)

            out_sb = out_pool.tile([P, T], f16, tag="o")
            for half in range(T // HALF):
                h0 = half * HALF
                y3 = y_pool.tile([P, HALF], f16, tag="y3")
                nc.scalar.activation(
                    y3[:],
                    xt[:, h0 + K - 1 : h0 + K - 1 + HALF],
                    mybir.ActivationFunctionType.Identity,
                    bias=wpack_sb[:, 2 * cb + 1 : 2 * cb + 2],
                    scale=wpack_sb[:, 2 * cb : 2 * cb + 1],
                )
                ps = psum_pool.tile([P, HALF], mybir.dt.float32, tag="ps")
                for k in range(3):
                    s0 = (cb * 3 + k) * 32
                    for q in range(HALF // TT):
                        t0 = h0 + q * TT
                        for g in range(4):
                            p0 = 32 * g
                            nc.tensor.matmul(
                                ps[p0 : p0 + 32, q * TT : (q + 1) * TT],
                                dpack_sb[p0 : p0 + 32, s0 : s0 + 32],
                                xt[p0 : p0 + 32, t0 + k : t0 + k + TT],
                                start=(k == 0),
                                stop=(k == 2),
                                skip_group_check=True,
                                tile_position=(p0, p0),
                            )
                nc.vector.tensor_tensor(
                    out_sb[:, h0 : h0 + HALF],
                    ps[:],
                    y3[:],
                    mybir.AluOpType.add,
                )
                # Store per half on sync HWDGE (SP is otherwise idle).  A
                # store's completion consumer is the out-buf reuse 4 blocks
                # (~28 us) later, so the optimistic HWDGE completion inc is
                # harmless here, and same-lane stores are 4 blocks apart,
                # gated behind the waiting DVE write itself.
                nc.sync.dma_start(
                    o_d[c0 : c0 + P, h0 : h0 + HALF],
                    out_sb[:, h0 : h0 + HALF],
                )

    nc.compile()
    return nc


def _build_v7() -> bass.Bass:
    """Previous baseline (full-array diag matmuls), kept for A/B."""
    f16 = mybir.dt.float16
    nc = bacc.Bacc("TRN2", target_bir_lowering=False, debug=False)

    x_d = nc.dram_tensor("x", [C, T], f16, kind="ExternalInput")
    w_d = nc.dram_tensor("w", [C, K], mybir.dt.float32, kind="ExternalInput")
    b_d = nc.dram_tensor("b", [C, 1], mybir.dt.float32, kind="ExternalInput")
    o_d = nc.dram_tensor("out", [C, T], f16, kind="ExternalOutput")
    ident_d = nc.inline_tensor(np.eye(P, dtype=np.float32), "ident")

    with tile.TileContext(nc) as tc, ExitStack() as ctx:
        id_pool = ctx.enter_context(tc.tile_pool(name="id", bufs=1))
        x_pool = ctx.enter_context(tc.tile_pool(name="x", bufs=4))
        out_pool = ctx.enter_context(tc.tile_pool(name="o", bufs=4))
        wb_pool = ctx.enter_context(tc.tile_pool(name="wb", bufs=3))
        lhs_pool = ctx.enter_context(tc.tile_pool(name="lhs", bufs=12))
        y_pool = ctx.enter_context(tc.tile_pool(name="y", bufs=3))
        psum_pool = ctx.enter_context(
            tc.tile_pool(name="ps", bufs=2, space="PSUM")
        )

        id_sb = id_pool.tile([P, P], mybir.dt.float32, tag="ident")
        nc.sync.dma_start(id_sb[:], ident_d[:])

        for cb in range(CB):
            c0 = cb * P

            w_sb = wb_pool.tile([P, K], mybir.dt.float32, tag="w")
            nc.gpsimd.dma_start(w_sb[:], w_d[c0 : c0 + P, :])
            bias_sb = wb_pool.tile([P, 1], mybir.dt.float32, tag="bias")
            nc.gpsimd.dma_start(bias_sb[:], b_d[c0 : c0 + P, :])

            xt = x_pool.tile([P, T + K - 1], f16, tag="x")
            nc.vector.memset(xt[:, 0 : K - 1], 0)
            nc.sync.dma_start(xt[:, K - 1 : T + K - 1], x_d[c0 : c0 + P, :])

            lhs = []
            for k in range(3):
                lk = lhs_pool.tile([P, P], f16, tag="lhs")
                nc.scalar.mul(lk[:], id_sb[:], w_sb[:, k : k + 1])
                lhs.append(lk)

            y3 = y_pool.tile([P, T], mybir.dt.float32, tag="y3")
            out_sb = out_pool.tile([P, T], mybir.dt.float16, tag="o")
            for half in range(T // HALF):
                ps = psum_pool.tile([P, HALF], mybir.dt.float32, tag="ps")
                h0 = half * HALF
                nc.scalar.activation(
                    y3[:, h0 : h0 + HALF],
                    xt[:, h0 + K - 1 : h0 + K - 1 + HALF],
                    mybir.ActivationFunctionType.Identity,
                    bias=bias_sb[:],
                    scale=w_sb[:, 3:4],
                )
                for k in range(3):
                    for q in range(HALF // TT):
                        t0 = h0 + q * TT
                        nc.tensor.matmul(
                            ps[:, q * TT : (q + 1) * TT],
                            lhs[k][:],
                            xt[:, t0 + k : t0 + k + TT],
                            start=(k == 0),
                            stop=(k == 2),
                            skip_group_check=True,
                        )
                nc.vector.tensor_tensor(
                    out_sb[:, h0 : h0 + HALF],
                    ps[:],
                    y3[:, h0 : h0 + HALF],
                    mybir.AluOpType.add,
                )
                nc.scalar.dma_start(
                    o_d[c0 : c0 + P, h0 : h0 + HALF],
                    out_sb[:, h0 : h0 + HALF],
                )

    nc.compile()
    return nc


def _get_program(mode: str) -> bass.Bass:
    if mode not in _PROGRAM_CACHE:
        _PROGRAM_CACHE[mode] = _build_t3() if mode == "t3" else _build_v7()
    return _PROGRAM_CACHE[mode]


def _host_pack(weight: np.ndarray, bias: np.ndarray):
    """Pack diag lhsT strips (fp16) and tap3 scale/bias (fp32)."""
    w4 = np.ascontiguousarray(weight[:, 0, :]).astype(np.float32)  # [C, K]
    w16 = w4.astype(np.float16)
    dpack = np.zeros((P, CB * 3 * 32), dtype=np.float16)
    j = np.arange(32)
    for cb in range(CB):
        for k in range(3):
            col0 = (cb * 3 + k) * 32
            for g in range(4):
                dpack[32 * g + j, col0 + j] = w16[cb * P + 32 * g + j, k]
    wpack = np.zeros((P, CB * 2), dtype=np.float32)
    for cb in range(CB):
        wpack[:, 2 * cb] = w4[cb * P : (cb + 1) * P, 3]
        wpack[:, 2 * cb + 1] = bias[cb * P : (cb + 1) * P]
    return dpack, wpack


def kernel(x: np.ndarray, weight: np.ndarray, bias: np.ndarray) -> np.ndarray:
    global LAST_EXEC_NS, LAST_RESULTS

    x = np.asarray(x, dtype=np.float32)
    weight = np.asarray(weight, dtype=np.float32)
    bias = np.asarray(bias, dtype=np.float32)

    # [B, T, C] -> [B, C, T] so time is contiguous per channel row.
    xt = x.transpose(0, 2, 1).astype(np.float16)

    nc = _get_program(MODE)
    if MODE == "t3":
        dpack, wpack = _host_pack(weight, bias)
        in_maps = [
            {"x": xt[b], "dpack": dpack, "wpack": wpack} for b in range(B)
        ]
    else:
        w4 = np.ascontiguousarray(weight[:, 0, :])
        b2 = np.ascontiguousarray(bias.reshape(C, 1))
        in_maps = [{"x": xt[b], "w": w4, "b": b2} for b in range(B)]

    trace = bool(os.environ.get("KERNEL_PROFILE"))
    if trace:
        _setup_profiling()
    res = run_bass_kernel_spmd(
        nc,
        in_maps,
        list(range(N_CORES)),
        trace=trace,
        tmpdir=os.environ.get("KERNEL_PROFILE_DIR") or None,
    )
    LAST_EXEC_NS = res.exec_time_ns
    LAST_RESULTS = res

    out = np.empty((B, T, C), dtype=np.float32)
    for b in range(B):
        out[b] = res.results[b]["out"].T.astype(np.float32)
    return out


# revision 12
# speedup vs baseline: 1.0246x; 1.0246x over previous
"""Causal depthwise Conv1d (B=8, T=4096, C=2048, K=4), fp32, on 8 NeuronCores.

Mode "t3" (default): batch-parallel across 8 cores, fp16 device I/O
(host casts + transposes to [B, C, T]).  Per 128-channel block:

  - PE: taps 0..2 as 32x32 *tiled* diagonal matmuls.  Each 128-wide diag
    matmul is split into its 4 nonzero 32x32 diagonal tiles via
    tile_position=(32g, 32g).  The 4 sub-arrays stream concurrently and
    each LDWEIGHTS is only 32 columns (~27 ns) and can be pulled ahead
    across row groups, removing the ~100 ns/MM serialized weight-reload
    tax of the full-array version (v7: 325 ns/MM -> target ~220 ns/MM
    equivalent).
  - The diagonal lhsT tiles are precomputed on HOST into a packed
    [128, 16*3*32] fp16 tensor (one 32-col strip per (block, tap) with
    the 4 diag tiles stacked per partition group) and DMA'd once --
    this removes the 48 ACT builds of v7.
  - ACT: tap 3 + bias via the activation affine in fp16-out mode
    (2x the fp32 rate), per 2048-col half.
  - DVE: out = psum + y3 (tensor_tensor, fp16 out), per half.
  - tap-3 scale/bias come from a single upfront packed [128, 32] fp32
    DMA (wpack) instead of 32 tiny per-block DMAs.
  - input x DMA per block (1 MB) on the sync HWDGE queue; output store
    per block (1 MB) on the scalar HWDGE queue.

Numerics identical to v7 (fp16 taps, exact PE products, fp32 PSUM)
except y3 is rounded to fp16 before the final add: absmax/scale
~1e-3 vs the 2e-2 budget.

Mode "v7" (previous baseline, kept for A/B): full-array diag matmuls,
ACT-built lhsT, fp32 y3.  ~125.5 us HW; PE-bound on serialized
LDWEIGHTS; also shows an intermittent sparse-error race (~1 in 2 runs
observed absmax/scale 1.6e-1 from a handful of elements).
"""

import os
from contextlib import ExitStack

import numpy as np

import concourse.bacc as bacc
import concourse.bass as bass
import concourse.mybir as mybir
import concourse.tile as tile
from concourse.bass_utils import run_bass_kernel_spmd

B, T, C, K = 8, 4096, 2048, 4
P = 128                 # partitions per channel block
CB = C // P             # 16 channel blocks
TT = 512                # free-dim cols per matmul (one PSUM bank)
HALF = 2048             # free elements per PSUM tile (4 banks)
N_CORES = 8

MODE = os.environ.get("KERNEL_MODE", "t3")

LAST_EXEC_NS = None
LAST_RESULTS = None

_PROGRAM_CACHE = {}
_PROFILING_READY = False


def _setup_profiling():
    """Register the axon NTFF profile hook (the image lacks
    antenv.axon_hooks, so shim it into sys.modules) and neuter the S3
    artifact upload."""
    global _PROFILING_READY
    if _PROFILING_READY:
        return
    import sys
    import types

    if "antenv.axon_hooks" not in sys.modules:
        mod = types.ModuleType("antenv.axon_hooks")
        mod._hook = None

        def set_axon_ntff_profile_hook(h):
            mod._hook = h

        def get_axon_ntff_profile_hook():
            return mod._hook

        mod.set_axon_ntff_profile_hook = set_axon_ntff_profile_hook
        mod.get_axon_ntff_profile_hook = get_axon_ntff_profile_hook
        sys.modules["antenv.axon_hooks"] = mod
        import antenv

        antenv.axon_hooks = mod

    from antenv.axon_hooks import (
        get_axon_ntff_profile_hook,
        set_axon_ntff_profile_hook,
    )

    if get_axon_ntff_profile_hook() is None:
        from trn_agent_boot.trn_boot import _ntff_profile_via_ctypes

        set_axon_ntff_profile_hook(
            _ntff_profile_via_ctypes("/opt/axon/libaxon_pjrt.so")
        )

    import concourse.bass_utils as bu

    bu.upload_artifacts = lambda tmpdir: str(tmpdir)
    _PROFILING_READY = True


def _build_t3() -> bass.Bass:
    f16 = mybir.dt.float16
    nc = bacc.Bacc("TRN2", target_bir_lowering=False, debug=False)

    x_d = nc.dram_tensor("x", [C, T], f16, kind="ExternalInput")
    dpack_d = nc.dram_tensor(
        "dpack", [P, CB * 3 * 32], f16, kind="ExternalInput"
    )
    wpack_d = nc.dram_tensor(
        "wpack", [P, CB * 2], mybir.dt.float32, kind="ExternalInput"
    )
    o_d = nc.dram_tensor("out", [C, T], f16, kind="ExternalOutput")

    with tile.TileContext(nc) as tc, ExitStack() as ctx:
        const_pool = ctx.enter_context(tc.tile_pool(name="const", bufs=1))
        x_pool = ctx.enter_context(tc.tile_pool(name="x", bufs=6))
        out_pool = ctx.enter_context(tc.tile_pool(name="o", bufs=4))
        y_pool = ctx.enter_context(tc.tile_pool(name="y", bufs=3))
        psum_pool = ctx.enter_context(
            tc.tile_pool(name="ps", bufs=2, space="PSUM")
        )

        # dpack/wpack ride the sync HWDGE queue: the SP engine reaches its
        # first DMA right after the preamble barrier, while ACT is blocked
        # ~8 us by LoadActFuncSet table loads.
        dpack_sb = const_pool.tile([P, CB * 3 * 32], f16, tag="dpack")
        nc.sync.dma_start(dpack_sb[:], dpack_d[:])
        wpack_sb = const_pool.tile([P, CB * 2], mybir.dt.float32, tag="wpack")
        nc.sync.dma_start(wpack_sb[:], wpack_d[:])

        for cb in range(CB):
            c0 = cb * P

            # Input x rides SWDGE (gpsimd): its completion increment is
            # chained behind the data writes (write-after-write descriptor),
            # so consumers can't wake before the bytes land.  The HWDGE
            # completion inc was observed to overtake the last in-flight
            # SBUF writes by ~100ns under port contention, letting PE/ACT
            # read the first columns of one partition stale (~1 run in 2,
            # always output t=0..3 of one channel).
            xt = x_pool.tile([P, T + K - 1], f16, tag="x")
            if cb == 0:
                # Split the first block's load so half-0 consumers start
                # ~2.5 us earlier (pipeline fill).
                nc.gpsimd.dma_start(
                    xt[:, K - 1 : K - 1 + HALF], x_d[c0 : c0 + P, 0:HALF]
                )
                nc.gpsimd.dma_start(
                    xt[:, K - 1 + HALF : T + K - 1], x_d[c0 : c0 + P, HALF:T]
                )
            else:
                nc.gpsimd.dma_start(
                    xt[:, K - 1 : T + K - 1], x_d[c0 : c0 + P, :]
                )
            nc.vector.memset(xt[:, 0 : K - 1], 0)

            out_sb = out_pool.tile([P, T], f16, tag="o")
            for half in range(T // HALF):
                h0 = half * HALF
                # Two chunks, head chunk LAST: both DGE paths show a
                # ~100-300 ns window where the DMA completion increment
                # becomes visible before the final SBUF data writes of a
                # lagging SDMA engine; every observed corruption was in the
                # first ~4 columns of one partition row.  Reading the head
                # ~1.5 us after wake (instead of immediately) closes it.
                y3 = y_pool.tile([P, HALF], f16, tag="y3")
                for lo, hi in ((TT, HALF), (0, TT)):
                    nc.scalar.activation(
                        y3[:, lo:hi],
                        xt[:, h0 + K - 1 + lo : h0 + K - 1 + hi],
                        mybir.ActivationFunctionType.Identity,
                        bias=wpack_sb[:, 2 * cb + 1 : 2 * cb + 2],
                        scale=wpack_sb[:, 2 * cb : 2 * cb + 1],
                    )
                ps = psum_pool.tile([P, HALF], mybir.dt.float32, tag="ps")
                for k in range(3):
                    s0 = (cb * 3 + k) * 32
                    for q in (1, 2, 3, 0):  # head window (q=0) last
                        t0 = h0 + q * TT
                        for g in range(4):
                            p0 = 32 * g
                            nc.tensor.matmul(
                                ps[p0 : p0 + 32, q * TT : (q + 1) * TT],
                                dpack_sb[p0 : p0 + 32, s0 : s0 + 32],
                                xt[p0 : p0 + 32, t0 + k : t0 + k + TT],
                                start=(k == 0),
                                stop=(k == 2),
                                skip_group_check=True,
                                tile_position=(p0, p0),
                            )
                nc.vector.tensor_tensor(
                    out_sb[:, h0 : h0 + HALF],
                    ps[:],
                    y3[:],
                    mybir.AluOpType.add,
                )
                # Store per half on sync HWDGE (SP is otherwise idle).  A
                # store's completion consumer is the out-buf reuse 4 blocks
                # (~28 us) later, so the optimistic HWDGE completion inc is
                # harmless here, and same-lane stores are 4 blocks apart,
                # gated behind the waiting DVE write itself.
                nc.sync.dma_start(
                    o_d[c0 : c0 + P, h0 : h0 + HALF],
                    out_sb[:, h0 : h0 + HALF],
                )

    nc.compile()
    return nc


def _build_v7() -> bass.Bass:
    """Previous baseline (full-array diag matmuls), kept for A/B."""
    f16 = mybir.dt.float16
    nc = bacc.Bacc("TRN2", target_bir_lowering=False, debug=False)

    x_d = nc.dram_tensor("x", [C, T], f16, kind="ExternalInput")
    w_d = nc.dram_tensor("w", [C, K], mybir.dt.float32, kind="ExternalInput")
    b_d = nc.dram_tensor("b", [C, 1], mybir.dt.float32, kind="ExternalInput")
    o_d = nc.dram_tensor("out", [C, T], f16, kind="ExternalOutput")
    ident_d = nc.inline_tensor(np.eye(P, dtype=np.float32), "ident")

    with tile.TileContext(nc) as tc, ExitStack() as ctx:
        id_pool = ctx.enter_context(tc.tile_pool(name="id", bufs=1))
        x_pool = ctx.enter_context(tc.tile_pool(name="x", bufs=4))
        out_pool = ctx.enter_context(tc.tile_pool(name="o", bufs=4))
        wb_pool = ctx.enter_context(tc.tile_pool(name="wb", bufs=3))
        lhs_pool = ctx.enter_context(tc.tile_pool(name="lhs", bufs=12))
        y_pool = ctx.enter_context(tc.tile_pool(name="y", bufs=3))
        psum_pool = ctx.enter_context(
            tc.tile_pool(name="ps", bufs=2, space="PSUM")
        )

        id_sb = id_pool.tile([P, P], mybir.dt.float32, tag="ident")
        nc.sync.dma_start(id_sb[:], ident_d[:])

        for cb in range(CB):
            c0 = cb * P

            w_sb = wb_pool.tile([P, K], mybir.dt.float32, tag="w")
            nc.gpsimd.dma_start(w_sb[:], w_d[c0 : c0 + P, :])
            bias_sb = wb_pool.tile([P, 1], mybir.dt.float32, tag="bias")
            nc.gpsimd.dma_start(bias_sb[:], b_d[c0 : c0 + P, :])

            xt = x_pool.tile([P, T + K - 1], f16, tag="x")
            nc.vector.memset(xt[:, 0 : K - 1], 0)
            nc.sync.dma_start(xt[:, K - 1 : T + K - 1], x_d[c0 : c0 + P, :])

            lhs = []
            for k in range(3):
                lk = lhs_pool.tile([P, P], f16, tag="lhs")
                nc.scalar.mul(lk[:], id_sb[:], w_sb[:, k : k + 1])
                lhs.append(lk)

            y3 = y_pool.tile([P, T], mybir.dt.float32, tag="y3")
            out_sb = out_pool.tile([P, T], mybir.dt.float16, tag="o")
            for half in range(T // HALF):
                ps = psum_pool.tile([P, HALF], mybir.dt.float32, tag="ps")
                h0 = half * HALF
                nc.scalar.activation(
                    y3[:, h0 : h0 + HALF],
                    xt[:, h0 + K - 1 : h0 + K - 1 + HALF],
                    mybir.ActivationFunctionType.Identity,
                    bias=bias_sb[:],
                    scale=w_sb[:, 3:4],
                )
                for k in range(3):
                    for q in range(HALF // TT):
                        t0 = h0 + q * TT
                        nc.tensor.matmul(
                            ps[:, q * TT : (q + 1) * TT],
                            lhs[k][:],
                            xt[:, t0 + k : t0 + k + TT],
                            start=(k == 0),
                            stop=(k == 2),
                            skip_group_check=True,
                        )
                nc.vector.tensor_tensor(
                    out_sb[:, h0 : h0 + HALF],
                    ps[:],
                    y3[:, h0 : h0 + HALF],
                    mybir.AluOpType.add,
                )
                nc.scalar.dma_start(
                    o_d[c0 : c0 + P, h0 : h0 + HALF],
                    out_sb[:, h0 : h0 + HALF],
                )

    nc.compile()
    return nc


def _get_program(mode: str) -> bass.Bass:
    if mode not in _PROGRAM_CACHE:
        _PROGRAM_CACHE[mode] = _build_t3() if mode == "t3" else _build_v7()
    return _PROGRAM_CACHE[mode]


def _host_pack(weight: np.ndarray, bias: np.ndarray):
    """Pack diag lhsT strips (fp16) and tap3 scale/bias (fp32)."""
    w4 = np.ascontiguousarray(weight[:, 0, :]).astype(np.float32)  # [C, K]
    w16 = w4.astype(np.float16)
    dpack = np.zeros((P, CB * 3 * 32), dtype=np.float16)
    j = np.arange(32)
    for cb in range(CB):
        for k in range(3):
            col0 = (cb * 3 + k) * 32
            for g in range(4):
                dpack[32 * g + j, col0 + j] = w16[cb * P + 32 * g + j, k]
    wpack = np.zeros((P, CB * 2), dtype=np.float32)
    for cb in range(CB):
        wpack[:, 2 * cb] = w4[cb * P : (cb + 1) * P, 3]
        wpack[:, 2 * cb + 1] = bias[cb * P : (cb + 1) * P]
    return dpack, wpack


def kernel(x: np.ndarray, weight: np.ndarray, bias: np.ndarray) -> np.ndarray:
    global LAST_EXEC_NS, LAST_RESULTS

    x = np.asarray(x, dtype=np.float32)
    weight = np.asarray(weight, dtype=np.float32)
    bias = np.asarray(bias, dtype=np.float32)

    # [B, T, C] -> [B, C, T] so time is contiguous per channel row.
    xt = x.transpose(0, 2, 1).astype(np.float16)

    nc = _get_program(MODE)
    if MODE == "t3":
        dpack, wpack = _host_pack(weight, bias)
        in_maps = [
            {"x": xt[b], "dpack": dpack, "wpack": wpack} for b in range(B)
        ]
    else:
        w4 = np.ascontiguousarray(weight[:, 0, :])
        b2 = np.ascontiguousarray(bias.reshape(C, 1))
        in_maps = [{"x": xt[b], "w": w4, "b": b2} for b in range(B)]

    trace = bool(os.environ.get("KERNEL_PROFILE"))
    if trace:
        _setup_profiling()
    res = run_bass_kernel_spmd(
        nc,
        in_maps,
        list(range(N_CORES)),
        trace=trace,
        tmpdir=os.environ.get("KERNEL_PROFILE_DIR") or None,
    )
    LAST_EXEC_NS = res.exec_time_ns
    LAST_RESULTS = res

    out = np.empty((B, T, C), dtype=np.float32)
    for b in range(B):
        out[b] = res.results[b]["out"].T.astype(np.float32)
    return out
